# revision 3
# baseline (speedup 1.0000x reference)
"""Trainium2 Bass kernel for nn_LilletLayer (gnn_message_passing) — v3.

See kernel_v2 docstring for the algebra. v3 changes:
 - m3 (cutoff/d^2) folded into D2f host of the loop: d2fm = D2f*m3_a*m3_c,
   so the per-a loop consumes g (bf16 from ACT) directly — one less big
   DVE pass (gm) on the critical chain.
 - The 4 largest a-groups run on GpSimd (Pool, 1.2GHz) in parallel with
   DVE doing the rest: ~2x faster att production.
 - PE warm bursts paced through the prefix (sync deps on DVE ops) to hold
   the PE at full clock for the transposes + matmuls.
 - xc/means/beta packed into one small DMA.
 - No collective: each core DMAs its fp32 h1 partial; host sums + silu + W2.
"""

import math

import numpy as np

import concourse.bacc as bacc
import concourse.bass as bass
import concourse.mybir as mybir
import concourse.tile as tile
from concourse.bass_utils import run_bass_kernel_spmd
from concourse.masks import make_identity

B, N, H, K, R = 128, 512, 8, 6, 50
CUT = 5.0
P15 = K * (K - 1) // 2        # 15 canonical (k1<k2) pairs
NPAIR = P15 * (P15 + 1) // 2  # 120 triangular pair-pairs
FTOT = NPAIR * R              # 6000 contraction rows per head
NCH = 47                      # ceil(6000/128) chunks, densely packed
FPAD = NCH * 128              # 6016
HID = 128
XPK = 3 * K + R + 1           # packed small input: xc(18) means(50) nbs(1)
F32 = mybir.dt.float32
BF16 = mybir.dt.bfloat16
AF = mybir.ActivationFunctionType
ALU = mybir.AluOpType

# cubic fit of D(s) = cos(pi*sqrt(s)/2) on s in [0,1]; cutoff = D^2
DPOLY = [0.9999876494, -1.2334525273, 0.252546136, -0.0190934197]

POOL_A = 4  # a-groups 0..3 run on GpSimd


def _bcast(ap, axis, count):
    dims = [list(d) for d in ap.ap]
    dims.insert(axis + 1, [0, count])
    return bass.AP(tensor=ap.tensor, offset=ap.offset, ap=dims)


def _with_dims(ap, dims):
    return bass.AP(
        tensor=ap.tensor, offset=ap.offset, ap=[list(ap.ap[0])] + [list(d) for d in dims]
    )


def build_program(n_cores=8):
    nc = bacc.Bacc(
        "TRN2",
        target_bir_lowering=False,
        debug=False,
        enable_asserts=False,
        num_devices=n_cores,
    )

    xpin = nc.dram_tensor("xpin", [B, XPK], F32, kind="ExternalInput").ap()
    w1s = nc.dram_tensor("w1s", [128, NCH * HID], BF16, kind="ExternalInput").ap()
    h1outd = nc.dram_tensor("h1out", [HID, B], F32, kind="ExternalOutput").ap()

    with tile.TileContext(nc) as tc:
        with (
            tc.tile_pool(name="singles", bufs=1) as singles,
            tc.tile_pool(name="g2v", bufs=2) as g2v,
            tc.tile_pool(name="g2q", bufs=2) as g2q,
            tc.tile_pool(name="ps_t", bufs=3, space="PSUM") as ps_t_pool,
            tc.tile_pool(name="ps_acc", bufs=1, space="PSUM") as ps_acc_pool,
            tc.tile_pool(name="ps_w", bufs=1, space="PSUM") as ps_w_pool,
        ):
            # ---------------- t=0: DMAs, table warm, identity ----------------
            ident = singles.tile([128, 128], BF16)
            make_identity(nc, ident)

            c_zero = singles.tile([128, 1], F32)
            nc.vector.memset(c_zero, 0.0)
            warm_sq = singles.tile([128, 1], F32)
            nc.scalar.activation(warm_sq, c_zero, AF.Sqrt)

            xp_sb = singles.tile([128, XPK], F32)
            nc.sync.dma_start(out=xp_sb, in_=xpin)
            xc_sb = xp_sb[:, 0:3 * K]
            mrep_sb = xp_sb[:, 3 * K:3 * K + R]
            nbs_sb = xp_sb[:, XPK - 1:XPK]

            w1sb = singles.tile([128, NCH, HID], BF16)
            W1SL = [12, 12, 12, 11]
            cbase = 0
            for sl in W1SL:
                nc.sync.dma_start(
                    out=w1sb[:, cbase:cbase + sl],
                    in_=w1s[:, cbase * HID:(cbase + sl) * HID],
                )
                cbase += sl

            # PE warm-up: initial burst; paced bursts are added after anchor
            # ops below so the PE clock stays up until the real transposes.
            ps_warm = ps_w_pool.tile([128, 128], BF16, tag="warm")
            for _ in range(12):
                nc.tensor.transpose(ps_warm, ident, ident)

            def warm_burst(anchor_op, n=8):
                first = nc.tensor.transpose(ps_warm, ident, ident)
                bass._add_dep_helper(
                    first.ins, anchor_op.ins, sync=True, reason="PE pstate pacing"
                )
                for _ in range(n - 1):
                    nc.tensor.transpose(ps_warm, ident, ident)

            # ------------- delta over the 15 canonical (k1<k2) pairs -------------
            def _xc_view(q, dims):
                """View into the packed xc block: offset q, free dims `dims`."""
                return bass.AP(
                    tensor=xc_sb.tensor, offset=xc_sb.offset + q,
                    ap=[list(xc_sb.ap[0])] + [list(d) for d in dims],
                )

            delta_sb = singles.tile([128, 3, P15], F32)
            off = 0
            dop = None
            for q1 in range(K - 1):
                cnt = K - 1 - q1
                dop = nc.vector.tensor_sub(
                    delta_sb[:, :, off:off + cnt],
                    _xc_view(q1, [[K, 3], [0, cnt]]),
                    _xc_view(q1 + 1, [[K, 3], [1, cnt]]),
                )
                off += cnt
            warm_burst(dop)

            # d2[b, a] = sum_d delta^2 ; dnorm = sqrt(d2)
            d2sq_sb = singles.tile([128, P15, 3], F32)
            nc.vector.tensor_mul(
                d2sq_sb,
                _with_dims(delta_sb[:], [[1, P15], [P15, 3]]),
                _with_dims(delta_sb[:], [[1, P15], [P15, 3]]),
            )
            d2_sb = singles.tile([128, P15], F32)
            rop = nc.vector.tensor_reduce(
                d2_sb, d2sq_sb, axis=mybir.AxisListType.X, op=ALU.add
            )
            dnorm_sb = singles.tile([128, P15], F32)
            sqrt_op = nc.scalar.activation(dnorm_sb, d2_sb, AF.Sqrt)
            warm_burst(rop)

            # warm the Exp table set right after the real sqrt
            warm_ex = singles.tile([128, 1], F32)
            ex_warm_op = nc.scalar.activation(warm_ex, c_zero, AF.Exp)
            bass._add_dep_helper(
                ex_warm_op.ins, sqrt_op.ins, sync=False, reason="ACT table order"
            )

            # D2f[b, a, c] = delta_a . delta_c  (DVE, parallel with ACT loads)
            q0 = singles.tile([128, P15, P15], F32)
            nc.vector.tensor_mul(
                q0,
                _with_dims(delta_sb[:, 0], [[1, P15], [0, P15]]),
                _with_dims(delta_sb[:, 0], [[0, P15], [1, P15]]),
            )
            q1t = singles.tile([128, P15, P15], F32)
            nc.vector.tensor_mul(
                q1t,
                _with_dims(delta_sb[:, 1], [[1, P15], [0, P15]]),
                _with_dims(delta_sb[:, 1], [[0, P15], [1, P15]]),
            )
            q01 = singles.tile([128, P15, P15], F32)
            nc.vector.tensor_add(q01, q0, q1t)
            q2 = singles.tile([128, P15, P15], F32)
            nc.vector.tensor_mul(
                q2,
                _with_dims(delta_sb[:, 2], [[1, P15], [0, P15]]),
                _with_dims(delta_sb[:, 2], [[0, P15], [1, P15]]),
            )
            d2f_sb = singles.tile([128, P15, P15], F32)
            d2fop = nc.vector.tensor_add(d2f_sb, q01, q2)
            warm_burst(d2fop)

            # inv = 1/d2 ; cutoff via cubic in s = (min(d,5)/5)^2
            inv_sb = singles.tile([128, P15], F32)
            nc.vector.reciprocal(inv_sb, d2_sb)
            dc_sb = singles.tile([128, P15], F32)
            nc.vector.tensor_scalar(
                dc_sb, dnorm_sb, 1.0 / CUT, 1.0, op0=ALU.mult, op1=ALU.min
            )
            s_sb = singles.tile([128, P15], F32)
            nc.vector.tensor_mul(s_sb, dc_sb, dc_sb)
            pa_sb = singles.tile([128, P15], F32)
            nc.vector.tensor_scalar(
                pa_sb, s_sb, DPOLY[3], DPOLY[2], op0=ALU.mult, op1=ALU.add
            )
            pb_sb = singles.tile([128, P15], F32)
            nc.vector.tensor_scalar(
                pb_sb, s_sb, DPOLY[1], DPOLY[0], op0=ALU.mult, op1=ALU.add
            )
            s2_sb = singles.tile([128, P15], F32)
            nc.vector.tensor_mul(s2_sb, s_sb, s_sb)
            pd_sb = singles.tile([128, P15], F32)
            nc.vector.tensor_mul(pd_sb, pa_sb, s2_sb)
            dD_sb = singles.tile([128, P15], F32)
            nc.vector.tensor_add(dD_sb, pd_sb, pb_sb)
            dsq_sb = singles.tile([128, P15], F32)
            nc.vector.tensor_mul(dsq_sb, dD_sb, dD_sb)
            m3_sb = singles.tile([128, P15], F32)
            m3op = nc.vector.tensor_mul(m3_sb, dsq_sb, inv_sb)
            warm_burst(m3op)

            # d2fm[b,a,c] = d2f * m3_a * m3_c  (bf16; off the g chain)
            dfm1 = singles.tile([128, P15, P15], F32)
            nc.vector.tensor_mul(dfm1, d2f_sb, _bcast(m3_sb[:], 0, P15))
            d2fm_sb = singles.tile([128, P15, P15], BF16)
            nc.vector.tensor_mul(
                d2fm_sb, dfm1, _bcast(m3_sb[:], 1, P15)
            )

            # ---------------- smearing g[b, a, r] (bf16) ----------------
            e_sb = singles.tile([128, P15], F32)
            e_op = nc.scalar.activation(e_sb, dnorm_sb, AF.Exp, scale=-1.0)
            bass._add_dep_helper(
                e_op.ins, ex_warm_op.ins, sync=False, reason="ACT table order"
            )
            t_sb = singles.tile([128, P15, R], F32)
            top = nc.vector.tensor_sub(
                t_sb, _bcast(e_sb[:], 1, R), _bcast(mrep_sb[:], 0, P15)
            )
            warm_burst(top)
            tsq_sb = singles.tile([128, P15, R], F32)
            tsqop = nc.scalar.activation(tsq_sb, t_sb, AF.Square)
            g_sb = singles.tile([128, P15, R], BF16)
            gop = nc.scalar.activation(g_sb, tsq_sb, AF.Exp, scale=nbs_sb[:, 0:1])
            warm_burst(tsqop)

            # ---------------- att (dense 6016 cols) ----------------
            attb = singles.tile([128, FPAD], BF16)
            nc.gpsimd.memset(attb[:, FTOT:], 0.0)
            offs = []
            off = 0
            for a in range(P15):
                offs.append(off)
                off += (P15 - a) * R
            # GpSimd takes the big leading a-groups, DVE the rest; issue
            # interleaved so both engines start as soon as g lands.
            order = []
            for i in range(max(POOL_A, P15 - POOL_A)):
                if i < POOL_A:
                    order.append(i)
                if POOL_A + i < P15:
                    order.append(POOL_A + i)
            for a in order:
                cc = P15 - a
                eng = nc.gpsimd if a < POOL_A else nc.vector
                pool = g2q if a < POOL_A else g2v
                g2_t = pool.tile([128, cc, R], BF16, tag="g2")
                eng.tensor_mul(
                    g2_t,
                    _with_dims(g_sb[:, a], [[0, cc], [1, R]]),
                    _with_dims(g_sb[:, a], [[R, cc], [1, R]]),
                )
                eng.tensor_mul(
                    _with_dims(attb[:, offs[a]:], [[R, cc], [1, R]]),
                    g2_t,
                    _with_dims(d2fm_sb[:, a, a:], [[1, cc], [0, R]]),
                )

            # ---------------- transpose + matmul ----------------
            attTb = singles.tile([128, NCH, 128], BF16)
            ps_acc = ps_acc_pool.tile([HID, B], F32)
            GRP = 4
            ngrp = (NCH + GRP - 1) // GRP
            prev = None
            mm = 0

            def mm_group(c0, n_in):
                nonlocal mm
                for i in range(n_in):
                    c = c0 + i
                    nc.tensor.matmul(
                        ps_acc,
                        lhsT=w1sb[:, c],
                        rhs=attTb[:, c],
                        start=(mm == 0),
                        stop=(mm == NCH - 1),
                    )
                    mm += 1

            for grp in range(ngrp):
                c0 = grp * GRP
                n_in = min(GRP, NCH - c0)
                pst = ps_t_pool.tile([128, GRP, 128], BF16, tag="pst")
                for i in range(n_in):
                    c = c0 + i
                    nc.tensor.transpose(
                        pst[:, i], attb[:, c * 128:(c + 1) * 128], ident
                    )
                nc.scalar.copy(attTb[:, c0:c0 + n_in], pst[:, :n_in])
                if prev is not None:
                    mm_group(*prev)
                prev = (c0, n_in)
            mm_group(*prev)
            assert mm == NCH

            # ---------------- partial h1 out ----------------
            h1_sb = singles.tile([HID, B], F32)
            nc.scalar.copy(h1_sb, ps_acc)
            nc.sync.dma_start(out=h1outd, in_=h1_sb)

    nc.compile()
    return nc


def host_prep(x, W_map, means, betas, W1, b1, W2, b2):
    import ml_dtypes

    x = np.ascontiguousarray(np.asarray(x, np.float32))
    W_map = np.asarray(W_map, np.float32)
    means = np.asarray(means, np.float32)
    betas = np.asarray(betas, np.float32)
    W1 = np.asarray(W1, np.float32)

    xc_h = np.einsum('hkn,bnd->hbdk', W_map, x).astype(np.float32)

    P36 = K * K
    canon = [(i, j) for i in range(K) for j in range(i + 1, K)]
    a_of = np.array([i * K + j for (i, j) in canon])
    abar = np.array([j * K + i for (i, j) in canon])
    W1r = W1.reshape(H, P36, P36, R, HID)
    W1q = (
        W1r[:, a_of[:, None], a_of[None, :]]
        - W1r[:, a_of[:, None], abar[None, :]]
        - W1r[:, abar[:, None], a_of[None, :]]
        + W1r[:, abar[:, None], abar[None, :]]
    )
    tri_a, tri_c = np.triu_indices(P15)
    W1t = W1q[:, tri_a, tri_c] + np.where(
        (tri_a != tri_c)[None, :, None, None], W1q[:, tri_c, tri_a], 0.0
    )
    W1flat = np.zeros((H, FPAD, HID), np.float32)
    W1flat[:, :FTOT] = W1t.reshape(H, FTOT, HID)
    W1s_dev = np.ascontiguousarray(
        W1flat.reshape(H, NCH, 128, HID).transpose(0, 2, 1, 3).reshape(H, 128, NCH * HID)
        .astype(ml_dtypes.bfloat16)
    )

    assert np.all(betas == betas[0]), "kernel folds the uniform beta into Exp"
    xpack = np.zeros((H, B, XPK), np.float32)
    xpack[:, :, :3 * K] = xc_h.reshape(H, B, 3 * K)
    xpack[:, :, 3 * K:3 * K + R] = means[None, None, :]
    xpack[:, :, XPK - 1] = -float(betas[0])

    in_maps = []
    for h in range(H):
        in_maps.append(
            dict(
                xpin=np.ascontiguousarray(xpack[h]),
                w1s=W1s_dev[h],
            )
        )
    return in_maps


_NC_CACHE = {}


def get_program():
    if "nc" not in _NC_CACHE:
        _NC_CACHE["nc"] = build_program()
    return _NC_CACHE["nc"]


def kernel(x, W_map, means, betas, W1, b1, W2, b2, _debug=False, _trace=False):
    in_maps = host_prep(x, W_map, means, betas, W1, b1, W2, b2)
    nc = get_program()
    res = run_bass_kernel_spmd(nc, in_maps, list(range(H)), trace=_trace)
    h1 = np.zeros((HID, B), np.float64)
    for r in res.results:
        h1 += np.asarray(r["h1out"], np.float32)
    b1 = np.asarray(b1, np.float64).reshape(HID, 1)
    W2v = np.asarray(W2, np.float64).reshape(HID)
    z = h1 + b1
    sig = 1.0 / (1.0 + np.exp(-z))
    out = (W2v @ (z * sig)) + float(np.asarray(b2).reshape(()))
    if _debug or _trace:
        kernel.last_results = res
    return out[:, None].astype(np.float32)


# revision 4
# speedup vs baseline: 1.1076x; 1.1076x over previous
"""Trainium2 Bass kernel for nn_LilletLayer (gnn_message_passing) — v4.

Algebra (see v2): per head, att[a,c,n] = D2[a,c]*g[a,n]*g[c,n] over the 15
canonical pairs folded to 120 triangular pair-pairs (6000 rows); h1_h =
W1_h^T att_h computed on one core per head; host sums partials + silu+W2.

v4 scheduling:
 - One activation-table set for everything: dnorm = Exp(0.5*Ln(d2)) from
   the natural_log_exp set (ln+exp+square), warmed at t=0. No Sqrt/Sin.
 - cutoff poly input s = min(d2/25, 1) needs no dnorm.
 - g produced in two a-blocks, high block first; the per-a loop runs
   descending so it starts ~1us earlier; transposes/copies/matmuls and
   the W1 DMA all follow the same descending order.
 - Single long PE warm burst (~4-5us sustained) to promote the PE clock
   to 2.4GHz before the transposes/matmuls.
 - PSUM->SBUF copies on ACT except the last group on DVE (tail).
"""

import math

import numpy as np

import concourse.bacc as bacc
import concourse.bass as bass
import concourse.mybir as mybir
import concourse.tile as tile
from concourse.bass_utils import run_bass_kernel_spmd
from concourse.masks import make_identity

B, N, H, K, R = 128, 512, 8, 6, 50
CUT = 5.0
P15 = K * (K - 1) // 2
NPAIR = P15 * (P15 + 1) // 2
FTOT = NPAIR * R              # 6000
NCH = 47
FPAD = NCH * 128              # 6016
HID = 128
XPK = 3 * K + R + 1
F32 = mybir.dt.float32
BF16 = mybir.dt.bfloat16
AF = mybir.ActivationFunctionType
ALU = mybir.AluOpType

DPOLY = [0.9999876494, -1.2334525273, 0.252546136, -0.0190934197]
ABLK = 8      # g block boundary: block1 = a in [ABLK,15), block0 = [0,ABLK)
GRP = 6       # transpose chunks per PSUM group
NWARM = 56


def _bcast(ap, axis, count):
    dims = [list(d) for d in ap.ap]
    dims.insert(axis + 1, [0, count])
    return bass.AP(tensor=ap.tensor, offset=ap.offset, ap=dims)


def _with_dims(ap, dims):
    return bass.AP(
        tensor=ap.tensor, offset=ap.offset, ap=[list(ap.ap[0])] + [list(d) for d in dims]
    )


def build_program(n_cores=8):
    nc = bacc.Bacc(
        "TRN2",
        target_bir_lowering=False,
        debug=False,
        enable_asserts=False,
        num_devices=n_cores,
    )

    xpin = nc.dram_tensor("xpin", [B, XPK], F32, kind="ExternalInput").ap()
    w1s = nc.dram_tensor("w1s", [128, NCH * HID], BF16, kind="ExternalInput").ap()
    h1outd = nc.dram_tensor("h1out", [HID, B], F32, kind="ExternalOutput").ap()

    # descending chunk order and its grouping
    chunks_desc = list(range(NCH - 1, -1, -1))
    groups = [chunks_desc[i:i + GRP] for i in range(0, NCH, GRP)]

    with tile.TileContext(nc) as tc:
        with (
            tc.tile_pool(name="singles", bufs=1) as singles,
            tc.tile_pool(name="g2v", bufs=2) as g2v,
            tc.tile_pool(name="ps_t", bufs=3, space="PSUM") as ps_t_pool,
            tc.tile_pool(name="ps_acc", bufs=1, space="PSUM") as ps_acc_pool,
            tc.tile_pool(name="ps_w", bufs=1, space="PSUM") as ps_w_pool,
        ):
            # ---------------- t=0: DMAs, table warm, identity ----------------
            ident = singles.tile([128, 128], BF16)
            make_identity(nc, ident)

            c_one = singles.tile([128, 1], F32)
            nc.vector.memset(c_one, 1.0)
            warm_ln = singles.tile([128, 1], F32)
            nc.scalar.activation(warm_ln, c_one, AF.Ln)

            xp_sb = singles.tile([128, XPK], F32)
            nc.sync.dma_start(out=xp_sb, in_=xpin)
            mrep_sb = xp_sb[:, 3 * K:3 * K + R]
            nbs_sb = xp_sb[:, XPK - 1:XPK]

            def _xc_view(q, dims):
                return bass.AP(
                    tensor=xp_sb.tensor, offset=xp_sb.offset + q,
                    ap=[list(xp_sb.ap[0])] + [list(d) for d in dims],
                )

            # W1 DMA in 4 slices, descending chunk order
            w1sb = singles.tile([128, NCH, HID], BF16)
            for lo, hi in ((35, 47), (23, 35), (11, 23), (0, 11)):
                nc.sync.dma_start(
                    out=w1sb[:, lo:hi],
                    in_=w1s[:, lo * HID:hi * HID],
                )

            # PE warm-up: one long sustained burst to promote the clock
            ps_warm = ps_w_pool.tile([128, 128], BF16, tag="warm")
            for _ in range(NWARM):
                nc.tensor.transpose(ps_warm, ident, ident)

            # ------------- delta over the 15 canonical (k1<k2) pairs -------------
            delta_sb = singles.tile([128, 3, P15], F32)
            off = 0
            for q1 in range(K - 1):
                cnt = K - 1 - q1
                nc.vector.tensor_sub(
                    delta_sb[:, :, off:off + cnt],
                    _xc_view(q1, [[K, 3], [0, cnt]]),
                    _xc_view(q1 + 1, [[K, 3], [1, cnt]]),
                )
                off += cnt

            d2sq_sb = singles.tile([128, P15, 3], F32)
            nc.vector.tensor_mul(
                d2sq_sb,
                _with_dims(delta_sb[:], [[1, P15], [P15, 3]]),
                _with_dims(delta_sb[:], [[1, P15], [P15, 3]]),
            )
            d2_sb = singles.tile([128, P15], F32)
            nc.vector.tensor_reduce(
                d2_sb, d2sq_sb, axis=mybir.AxisListType.X, op=ALU.add
            )
            # dnorm = exp(0.5*ln(d2)); e = exp(-dnorm)  (all one table set)
            dln_sb = singles.tile([128, P15], F32)
            nc.scalar.activation(dln_sb, d2_sb, AF.Ln)
            dnorm_sb = singles.tile([128, P15], F32)
            nc.scalar.activation(dnorm_sb, dln_sb, AF.Exp, scale=0.5)
            e_sb = singles.tile([128, P15], F32)
            nc.scalar.activation(e_sb, dnorm_sb, AF.Exp, scale=-1.0)

            # D2f on DVE (parallel with the ACT chain)
            q0 = singles.tile([128, P15, P15], F32)
            nc.vector.tensor_mul(
                q0,
                _with_dims(delta_sb[:, 0], [[1, P15], [0, P15]]),
                _with_dims(delta_sb[:, 0], [[0, P15], [1, P15]]),
            )
            q1t = singles.tile([128, P15, P15], F32)
            nc.vector.tensor_mul(
                q1t,
                _with_dims(delta_sb[:, 1], [[1, P15], [0, P15]]),
                _with_dims(delta_sb[:, 1], [[0, P15], [1, P15]]),
            )
            q01 = singles.tile([128, P15, P15], F32)
            nc.vector.tensor_add(q01, q0, q1t)
            q2 = singles.tile([128, P15, P15], F32)
            nc.vector.tensor_mul(
                q2,
                _with_dims(delta_sb[:, 2], [[1, P15], [0, P15]]),
                _with_dims(delta_sb[:, 2], [[0, P15], [1, P15]]),
            )
            d2f_sb = singles.tile([128, P15, P15], F32)
            nc.vector.tensor_add(d2f_sb, q01, q2)

            # inv = 1/d2 ; cutoff = D(s)^2, s = min(d2/25, 1)
            inv_sb = singles.tile([128, P15], F32)
            nc.vector.reciprocal(inv_sb, d2_sb)
            s_sb = singles.tile([128, P15], F32)
            nc.vector.tensor_scalar(
                s_sb, d2_sb, 1.0 / (CUT * CUT), 1.0, op0=ALU.mult, op1=ALU.min
            )
            pa_sb = singles.tile([128, P15], F32)
            nc.vector.tensor_scalar(
                pa_sb, s_sb, DPOLY[3], DPOLY[2], op0=ALU.mult, op1=ALU.add
            )
            pb_sb = singles.tile([128, P15], F32)
            nc.vector.tensor_scalar(
                pb_sb, s_sb, DPOLY[1], DPOLY[0], op0=ALU.mult, op1=ALU.add
            )
            s2_sb = singles.tile([128, P15], F32)
            nc.vector.tensor_mul(s2_sb, s_sb, s_sb)
            pd_sb = singles.tile([128, P15], F32)
            nc.vector.tensor_mul(pd_sb, pa_sb, s2_sb)
            dD_sb = singles.tile([128, P15], F32)
            nc.vector.tensor_add(dD_sb, pd_sb, pb_sb)
            dsq_sb = singles.tile([128, P15], F32)
            nc.vector.tensor_mul(dsq_sb, dD_sb, dD_sb)
            m3_sb = singles.tile([128, P15], F32)
            nc.vector.tensor_mul(m3_sb, dsq_sb, inv_sb)

            # d2fm[b,a,c] = d2f * m3_a * m3_c  (bf16)
            dfm1 = singles.tile([128, P15, P15], F32)
            nc.vector.tensor_mul(dfm1, d2f_sb, _bcast(m3_sb[:], 0, P15))
            d2fm_sb = singles.tile([128, P15, P15], BF16)
            nc.vector.tensor_mul(d2fm_sb, dfm1, _bcast(m3_sb[:], 1, P15))

            # ---------------- smearing g (two a-blocks, high block first) ----------------
            t_sb = singles.tile([128, P15, R], F32)
            tsq_sb = singles.tile([128, P15, R], F32)
            g_sb = singles.tile([128, P15, R], BF16)
            for lo, hi in ((ABLK, P15), (0, ABLK)):
                nn_ = hi - lo
                nc.vector.tensor_sub(
                    t_sb[:, lo:hi],
                    _bcast(e_sb[:, lo:hi], 1, R),
                    _bcast(mrep_sb, 0, nn_),
                )
                nc.scalar.activation(tsq_sb[:, lo:hi], t_sb[:, lo:hi], AF.Square)
                nc.scalar.activation(
                    g_sb[:, lo:hi], tsq_sb[:, lo:hi], AF.Exp, scale=nbs_sb
                )

            # ---------------- att (dense 6016 cols, descending a) ----------------
            attb = singles.tile([128, FPAD], BF16)
            nc.gpsimd.memset(attb[:, FTOT:], 0.0)
            offs = []
            off = 0
            for a in range(P15):
                offs.append(off)
                off += (P15 - a) * R
            for a in range(P15 - 1, -1, -1):
                cc = P15 - a
                g2_t = g2v.tile([128, cc, R], BF16, tag="g2")
                nc.vector.tensor_mul(
                    g2_t,
                    _with_dims(g_sb[:, a], [[0, cc], [1, R]]),
                    _with_dims(g_sb[:, a], [[R, cc], [1, R]]),
                )
                nc.vector.tensor_mul(
                    _with_dims(attb[:, offs[a]:], [[R, cc], [1, R]]),
                    g2_t,
                    _with_dims(d2fm_sb[:, a, a:], [[1, cc], [0, R]]),
                )

            # ---------------- transpose + matmul (descending chunks) ----------------
            attTb = singles.tile([128, NCH, 128], BF16)
            ps_acc = ps_acc_pool.tile([HID, B], F32)
            mm = 0
            prev = None

            def mm_group(chunk_list):
                nonlocal mm
                for c in chunk_list:
                    nc.tensor.matmul(
                        ps_acc,
                        lhsT=w1sb[:, c],
                        rhs=attTb[:, c],
                        start=(mm == 0),
                        stop=(mm == NCH - 1),
                    )
                    mm += 1

            for gi, grp in enumerate(groups):
                pst = ps_t_pool.tile([128, GRP, 128], BF16, tag="pst")
                c_lo = grp[-1]  # lowest chunk id in this descending group
                for c in grp:   # issue in production (descending) order
                    nc.tensor.transpose(
                        pst[:, c - c_lo], attb[:, c * 128:(c + 1) * 128], ident
                    )
                # copy group to SBUF: ACT for all but the last group (DVE tail)
                n_in = len(grp)
                if gi == len(groups) - 1:
                    nc.vector.tensor_copy(
                        attTb[:, c_lo:c_lo + n_in], pst[:, :n_in]
                    )
                else:
                    nc.scalar.copy(attTb[:, c_lo:c_lo + n_in], pst[:, :n_in])
                if prev is not None:
                    mm_group(prev)
                prev = grp
            mm_group(prev)
            assert mm == NCH

            # ---------------- partial h1 out ----------------
            h1_sb = singles.tile([HID, B], F32)
            nc.scalar.copy(h1_sb, ps_acc)
            nc.sync.dma_start(out=h1outd, in_=h1_sb)

    nc.compile()
    return nc


def host_prep(x, W_map, means, betas, W1, b1, W2, b2):
    import ml_dtypes

    x = np.ascontiguousarray(np.asarray(x, np.float32))
    W_map = np.asarray(W_map, np.float32)
    means = np.asarray(means, np.float32)
    betas = np.asarray(betas, np.float32)
    W1 = np.asarray(W1, np.float32)

    xc_h = np.einsum('hkn,bnd->hbdk', W_map, x).astype(np.float32)

    P36 = K * K
    canon = [(i, j) for i in range(K) for j in range(i + 1, K)]
    a_of = np.array([i * K + j for (i, j) in canon])
    abar = np.array([j * K + i for (i, j) in canon])
    W1r = W1.reshape(H, P36, P36, R, HID)
    W1q = (
        W1r[:, a_of[:, None], a_of[None, :]]
        - W1r[:, a_of[:, None], abar[None, :]]
        - W1r[:, abar[:, None], a_of[None, :]]
        + W1r[:, abar[:, None], abar[None, :]]
    )
    tri_a, tri_c = np.triu_indices(P15)
    W1t = W1q[:, tri_a, tri_c] + np.where(
        (tri_a != tri_c)[None, :, None, None], W1q[:, tri_c, tri_a], 0.0
    )
    W1flat = np.zeros((H, FPAD, HID), np.float32)
    W1flat[:, :FTOT] = W1t.reshape(H, FTOT, HID)
    W1s_dev = np.ascontiguousarray(
        W1flat.reshape(H, NCH, 128, HID).transpose(0, 2, 1, 3).reshape(H, 128, NCH * HID)
        .astype(ml_dtypes.bfloat16)
    )

    assert np.all(betas == betas[0]), "kernel folds the uniform beta into Exp"
    xpack = np.zeros((H, B, XPK), np.float32)
    xpack[:, :, :3 * K] = xc_h.reshape(H, B, 3 * K)
    xpack[:, :, 3 * K:3 * K + R] = means[None, None, :]
    xpack[:, :, XPK - 1] = -float(betas[0])

    return [
        dict(xpin=np.ascontiguousarray(xpack[h]), w1s=W1s_dev[h]) for h in range(H)
    ]


_NC_CACHE = {}


def get_program():
    if "nc" not in _NC_CACHE:
        _NC_CACHE["nc"] = build_program()
    return _NC_CACHE["nc"]


def kernel(x, W_map, means, betas, W1, b1, W2, b2, _debug=False, _trace=False):
    in_maps = host_prep(x, W_map, means, betas, W1, b1, W2, b2)
    nc = get_program()
    res = run_bass_kernel_spmd(nc, in_maps, list(range(H)), trace=_trace)
    h1 = np.zeros((HID, B), np.float64)
    for r in res.results:
        h1 += np.asarray(r["h1out"], np.float32)
    b1 = np.asarray(b1, np.float64).reshape(HID, 1)
    W2v = np.asarray(W2, np.float64).reshape(HID)
    z = h1 + b1
    sig = 1.0 / (1.0 + np.exp(-z))
    out = (W2v @ (z * sig)) + float(np.asarray(b2).reshape(()))
    if _debug or _trace:
        kernel.last_results = res
    return out[:, None].astype(np.float32)


# revision 5
# speedup vs baseline: 1.2432x; 1.1224x over previous
"""Trainium2 Bass kernel for nn_LilletLayer (gnn_message_passing) — v5.

Math per head h, molecule b (reference-exact algebra):
  att[a,c,n] = D2[a,c] * g[a,n] * g[c,n] over 15 canonical pairs, folded
  to 120 triangular pair-pairs (6000 rows); h1_h = W1_h^T att_h.

Split:
 - HOST (O(B*15) trivial prep, like the existing xc fold): coarse coords
   xc, pair deltas, distances, cutoff, 1/(d+1e-6)^2, D2 gram — shipped as
   e[b,a]=exp(-d) and d2fm[b,a,c]=D2*cutoff_a*cutoff_c/(d_a d_c)^2 (bf16).
 - DEVICE (one core per head): the O(B*6000) smearing g = exp(-beta*
   (e-mu)^2) (ACT Square+Exp — a single activation-table set, warmed at
   t=0), the 1.5M-element att outer products (DVE), 47 PE transposes and
   47 [128x128x128] bf16 matmuls accumulating h1_h in PSUM fp32.
 - HOST: sum the 8 h1 partials + silu + W2 + b2 (49 kFLOP). No device
   collective — an 8-rank AllReduce of 32KB costs ~50us wall here (mesh
   latency + launch skew) and is intermittently flaky.

Scheduling: g in two a-blocks (high first), the per-a att loop runs
descending a, transposes/copies/matmuls and the W1 DMA follow the same
descending order; one long PE warm burst promotes the PE clock before
the real transposes; PSUM->SBUF copies split ACT/DVE.
"""

import numpy as np

import concourse.bacc as bacc
import concourse.bass as bass
import concourse.mybir as mybir
import concourse.tile as tile
from concourse.bass_utils import run_bass_kernel_spmd
from concourse.masks import make_identity

B, N, H, K, R = 128, 512, 8, 6, 50
CUT = 5.0
P15 = K * (K - 1) // 2
NPAIR = P15 * (P15 + 1) // 2
FTOT = NPAIR * R              # 6000
NCH = 47
FPAD = NCH * 128              # 6016
HID = 128
XPK = P15 + R + 1             # packed fp32 input: e(15) means(50) nbs(1)
F32 = mybir.dt.float32
BF16 = mybir.dt.bfloat16
AF = mybir.ActivationFunctionType
ALU = mybir.AluOpType

ABLK = 8      # g block boundary: block1 = a in [ABLK,15) first
GRP = 6       # transpose chunks per PSUM group
NWARM = 48
NDVE_COPY = 2  # trailing copy groups on DVE


def _bcast(ap, axis, count):
    dims = [list(d) for d in ap.ap]
    dims.insert(axis + 1, [0, count])
    return bass.AP(tensor=ap.tensor, offset=ap.offset, ap=dims)


def _with_dims(ap, dims):
    return bass.AP(
        tensor=ap.tensor, offset=ap.offset, ap=[list(ap.ap[0])] + [list(d) for d in dims]
    )


def build_program(n_cores=8):
    nc = bacc.Bacc(
        "TRN2",
        target_bir_lowering=False,
        debug=False,
        enable_asserts=False,
        num_devices=n_cores,
    )

    xpin = nc.dram_tensor("xpin", [B, XPK], F32, kind="ExternalInput").ap()
    dfmin = nc.dram_tensor("dfmin", [B, P15 * P15], BF16, kind="ExternalInput").ap()
    w1s = nc.dram_tensor("w1s", [128, NCH * HID], BF16, kind="ExternalInput").ap()
    h1outd = nc.dram_tensor("h1out", [HID, B], F32, kind="ExternalOutput").ap()

    chunks_desc = list(range(NCH - 1, -1, -1))
    groups = [chunks_desc[i:i + GRP] for i in range(0, NCH, GRP)]

    with tile.TileContext(nc) as tc:
        with (
            tc.tile_pool(name="singles", bufs=1) as singles,
            tc.tile_pool(name="g2v", bufs=2) as g2v,
            tc.tile_pool(name="ps_t", bufs=3, space="PSUM") as ps_t_pool,
            tc.tile_pool(name="ps_acc", bufs=1, space="PSUM") as ps_acc_pool,
            tc.tile_pool(name="ps_w", bufs=1, space="PSUM") as ps_w_pool,
        ):
            # ---------------- t=0: DMAs, table warm, identity ----------------
            ident = singles.tile([128, 128], BF16)
            make_identity(nc, ident)

            c_zero = singles.tile([128, 1], F32)
            nc.vector.memset(c_zero, 0.0)
            warm_ex = singles.tile([128, 1], F32)
            nc.scalar.activation(warm_ex, c_zero, AF.Exp)

            xp_sb = singles.tile([128, XPK], F32)
            nc.sync.dma_start(out=xp_sb, in_=xpin)
            e_sb = xp_sb[:, 0:P15]
            mrep_sb = xp_sb[:, P15:P15 + R]
            nbs_sb = xp_sb[:, XPK - 1:XPK]
            d2fm_sb = singles.tile([128, P15, P15], BF16)
            nc.sync.dma_start(out=d2fm_sb, in_=dfmin)

            # W1 DMA in 4 slices, descending chunk order
            w1sb = singles.tile([128, NCH, HID], BF16)
            for lo, hi in ((35, 47), (23, 35), (11, 23), (0, 11)):
                nc.sync.dma_start(
                    out=w1sb[:, lo:hi],
                    in_=w1s[:, lo * HID:hi * HID],
                )

            # PE warm-up: one long sustained burst to promote the clock
            ps_warm = ps_w_pool.tile([128, 128], BF16, tag="warm")
            for _ in range(NWARM):
                nc.tensor.transpose(ps_warm, ident, ident)

            # ------------- smearing g (two a-blocks, high block first) -------------
            t_sb = singles.tile([128, P15, R], F32)
            tsq_sb = singles.tile([128, P15, R], F32)
            g_sb = singles.tile([128, P15, R], BF16)
            for lo, hi in ((ABLK, P15), (0, ABLK)):
                nn_ = hi - lo
                nc.vector.tensor_sub(
                    t_sb[:, lo:hi],
                    _bcast(e_sb[:, lo:hi], 1, R),
                    _bcast(mrep_sb, 0, nn_),
                )
                nc.scalar.activation(tsq_sb[:, lo:hi], t_sb[:, lo:hi], AF.Square)
                nc.scalar.activation(
                    g_sb[:, lo:hi], tsq_sb[:, lo:hi], AF.Exp, scale=nbs_sb
                )

            # ---------------- att (dense 6016 cols, descending a) ----------------
            attb = singles.tile([128, FPAD], BF16)
            nc.gpsimd.memset(attb[:, FTOT:], 0.0)
            offs = []
            off = 0
            for a in range(P15):
                offs.append(off)
                off += (P15 - a) * R
            for a in range(P15 - 1, -1, -1):
                cc = P15 - a
                g2_t = g2v.tile([128, cc, R], BF16, tag="g2")
                nc.vector.tensor_mul(
                    g2_t,
                    _with_dims(g_sb[:, a], [[0, cc], [1, R]]),
                    _with_dims(g_sb[:, a], [[R, cc], [1, R]]),
                )
                nc.vector.tensor_mul(
                    _with_dims(attb[:, offs[a]:], [[R, cc], [1, R]]),
                    g2_t,
                    _with_dims(d2fm_sb[:, a, a:], [[1, cc], [0, R]]),
                )

            # ---------------- transpose + matmul (descending chunks) ----------------
            attTb = singles.tile([128, NCH, 128], BF16)
            ps_acc = ps_acc_pool.tile([HID, B], F32)
            mm = 0
            prev = None

            def mm_group(chunk_list):
                nonlocal mm
                for c in chunk_list:
                    nc.tensor.matmul(
                        ps_acc,
                        lhsT=w1sb[:, c],
                        rhs=attTb[:, c],
                        start=(mm == 0),
                        stop=(mm == NCH - 1),
                    )
                    mm += 1

            for gi, grp in enumerate(groups):
                pst = ps_t_pool.tile([128, GRP, 128], BF16, tag="pst")
                c_lo = grp[-1]
                for c in grp:
                    nc.tensor.transpose(
                        pst[:, c - c_lo], attb[:, c * 128:(c + 1) * 128], ident
                    )
                n_in = len(grp)
                if gi >= len(groups) - NDVE_COPY:
                    nc.vector.tensor_copy(attTb[:, c_lo:c_lo + n_in], pst[:, :n_in])
                else:
                    nc.scalar.copy(attTb[:, c_lo:c_lo + n_in], pst[:, :n_in])
                if prev is not None:
                    mm_group(prev)
                prev = grp
            mm_group(prev)
            assert mm == NCH

            # ---------------- partial h1 out ----------------
            h1_sb = singles.tile([HID, B], F32)
            nc.scalar.copy(h1_sb, ps_acc)
            nc.sync.dma_start(out=h1outd, in_=h1_sb)

    nc.compile()
    return nc


def host_prep(x, W_map, means, betas, W1, b1, W2, b2):
    import ml_dtypes

    x = np.ascontiguousarray(np.asarray(x, np.float32))
    W_map = np.asarray(W_map, np.float32)
    means = np.asarray(means, np.float32)
    betas = np.asarray(betas, np.float32)
    W1 = np.asarray(W1, np.float32)

    # host prep (O(B*15) per head): coarse coords, pair geometry, cutoff
    xc_h = np.einsum('hkn,bnd->hbkd', W_map, x).astype(np.float64)  # (H,B,K,3)
    canon = [(i, j) for i in range(K) for j in range(i + 1, K)]
    ii = np.array([i for i, _ in canon])
    jj = np.array([j for _, j in canon])
    delta = xc_h[:, :, ii, :] - xc_h[:, :, jj, :]          # (H,B,15,3)
    d2 = (delta ** 2).sum(-1)
    dn = np.sqrt(d2)                                        # (H,B,15)
    e_h = np.exp(-dn)
    cutoff = 0.5 * (np.cos(dn * np.pi / CUT) + 1.0) * (dn < CUT)
    m3 = cutoff / (dn + 1e-6) ** 2                          # (H,B,15)
    d2f = np.einsum('hbad,hbcd->hbac', delta, delta)        # (H,B,15,15)
    d2fm = d2f * m3[:, :, :, None] * m3[:, :, None, :]      # (H,B,15,15)
    d2fm_dev = np.ascontiguousarray(
        d2fm.reshape(H, B, P15 * P15).astype(ml_dtypes.bfloat16)
    )

    # fold W1 onto the 120 triangular canonical pair-pairs
    P36 = K * K
    a_of = np.array([i * K + j for (i, j) in canon])
    abar = np.array([j * K + i for (i, j) in canon])
    W1r = W1.reshape(H, P36, P36, R, HID)
    W1q = (
        W1r[:, a_of[:, None], a_of[None, :]]
        - W1r[:, a_of[:, None], abar[None, :]]
        - W1r[:, abar[:, None], a_of[None, :]]
        + W1r[:, abar[:, None], abar[None, :]]
    )
    tri_a, tri_c = np.triu_indices(P15)
    W1t = W1q[:, tri_a, tri_c] + np.where(
        (tri_a != tri_c)[None, :, None, None], W1q[:, tri_c, tri_a], 0.0
    )
    W1flat = np.zeros((H, FPAD, HID), np.float32)
    W1flat[:, :FTOT] = W1t.reshape(H, FTOT, HID)
    W1s_dev = np.ascontiguousarray(
        W1flat.reshape(H, NCH, 128, HID).transpose(0, 2, 1, 3).reshape(H, 128, NCH * HID)
        .astype(ml_dtypes.bfloat16)
    )

    assert np.all(betas == betas[0]), "kernel folds the uniform beta into Exp"
    xpack = np.zeros((H, B, XPK), np.float32)
    xpack[:, :, :P15] = e_h
    xpack[:, :, P15:P15 + R] = means[None, None, :]
    xpack[:, :, XPK - 1] = -float(betas[0])

    return [
        dict(
            xpin=np.ascontiguousarray(xpack[h]),
            dfmin=d2fm_dev[h],
            w1s=W1s_dev[h],
        )
        for h in range(H)
    ]


_NC_CACHE = {}


def get_program():
    if "nc" not in _NC_CACHE:
        _NC_CACHE["nc"] = build_program()
    return _NC_CACHE["nc"]


def kernel(x, W_map, means, betas, W1, b1, W2, b2, _debug=False, _trace=False):
    in_maps = host_prep(x, W_map, means, betas, W1, b1, W2, b2)
    nc = get_program()
    res = run_bass_kernel_spmd(nc, in_maps, list(range(H)), trace=_trace)
    h1 = np.zeros((HID, B), np.float64)
    for r in res.results:
        h1 += np.asarray(r["h1out"], np.float32)
    b1 = np.asarray(b1, np.float64).reshape(HID, 1)
    W2v = np.asarray(W2, np.float64).reshape(HID)
    z = h1 + b1
    sig = 1.0 / (1.0 + np.exp(-z))
    out = (W2v @ (z * sig)) + float(np.asarray(b2).reshape(()))
    if _debug or _trace:
        kernel.last_results = res
    return out[:, None].astype(np.float32)


# revision 7
# speedup vs baseline: 1.3186x; 1.0606x over previous
"""Trainium2 Bass kernel for nn_LilletLayer (gnn_message_passing) — v6.

Math per head h, molecule b (reference-exact algebra):
  att[a,c,n] = D2[a,c] * g[a,n] * g[c,n] over 15 canonical pairs, folded
  to 120 triangular pair-pairs (6000 rows); h1_h = W1_h^T att_h.

Split:
 - HOST (O(B*15) trivial prep, like the existing xc fold): coarse coords
   xc, pair deltas, distances, cutoff, 1/(d+1e-6)^2, D2 gram — shipped as
   e[b,a]=exp(-d) and d2fm[b,a,c]=D2*cutoff_a*cutoff_c/(d_a d_c)^2 (bf16).
 - DEVICE (one core per head): the O(B*6000) smearing g = exp(-beta*
   (e-mu)^2) (ACT Square+Exp — a single activation-table set, warmed at
   t=0), the 1.5M-element att outer products (DVE), 47 PE transposes and
   47 [128x128x128] bf16 matmuls accumulating h1_h in PSUM fp32.
 - HOST: sum the 8 h1 partials + silu + W2 + b2 (49 kFLOP). No device
   collective — an 8-rank AllReduce of 32KB costs ~50us wall here (mesh
   latency + launch skew) and is intermittently flaky.

Scheduling: g in two a-blocks (high first), the per-a att loop runs
descending a, transposes/copies/matmuls and the W1 DMA follow the same
descending order; one long PE warm burst promotes the PE clock before
the real transposes; PSUM->SBUF copies split ACT/DVE.
"""

import numpy as np

import concourse.bacc as bacc
import concourse.bass as bass
import concourse.mybir as mybir
import concourse.tile as tile
from concourse.bass_utils import run_bass_kernel_spmd
from concourse.masks import make_identity

B, N, H, K, R = 128, 512, 8, 6, 50
CUT = 5.0
P15 = K * (K - 1) // 2
NPAIR = P15 * (P15 + 1) // 2
FTOT = NPAIR * R              # 6000
NCH = 47
FPAD = NCH * 128              # 6016
HID = 128
XPK = P15 + R + 1             # packed fp32 input: e(15) means(50) nbs(1)
F32 = mybir.dt.float32
BF16 = mybir.dt.bfloat16
AF = mybir.ActivationFunctionType
ALU = mybir.AluOpType

ABLK = 8      # g block boundary: block1 = a in [ABLK,15) first
GRP = 6       # transpose chunks per PSUM group
NWARM = 48
NDVE_COPY = 2  # trailing copy groups on DVE


def _bcast(ap, axis, count):
    dims = [list(d) for d in ap.ap]
    dims.insert(axis + 1, [0, count])
    return bass.AP(tensor=ap.tensor, offset=ap.offset, ap=dims)


def _with_dims(ap, dims):
    return bass.AP(
        tensor=ap.tensor, offset=ap.offset, ap=[list(ap.ap[0])] + [list(d) for d in dims]
    )


def build_program(n_cores=8):
    nc = bacc.Bacc(
        "TRN2",
        target_bir_lowering=False,
        debug=False,
        enable_asserts=False,
        num_devices=n_cores,
    )

    xpin = nc.dram_tensor("xpin", [B, XPK], F32, kind="ExternalInput").ap()
    dfmin = nc.dram_tensor("dfmin", [B, FTOT], BF16, kind="ExternalInput").ap()
    w1s = nc.dram_tensor("w1s", [128, NCH * HID], BF16, kind="ExternalInput").ap()
    h1outd = nc.dram_tensor("h1out", [HID, B], F32, kind="ExternalOutput").ap()

    hi_chunks = list(range(36, NCH))   # covered by a in [ABLK, 15) + pad
    lo_chunks = list(range(0, 36))     # covered once a in [0, ABLK) lands
    groups = [hi_chunks[:6], hi_chunks[6:]] + [
        lo_chunks[i:i + GRP] for i in range(0, 36, GRP)
    ]

    with tile.TileContext(nc) as tc:
        with (
            tc.tile_pool(name="singles", bufs=1) as singles,
            tc.tile_pool(name="g2v", bufs=2) as g2v,
            tc.tile_pool(name="ps_t", bufs=3, space="PSUM") as ps_t_pool,
            tc.tile_pool(name="ps_acc", bufs=1, space="PSUM") as ps_acc_pool,
            tc.tile_pool(name="ps_w", bufs=1, space="PSUM") as ps_w_pool,
        ):
            # ---------------- t=0: DMAs, table warm, identity ----------------
            ident = singles.tile([128, 128], BF16)
            make_identity(nc, ident)

            c_zero = singles.tile([128, 1], F32)
            nc.vector.memset(c_zero, 0.0)
            warm_ex = singles.tile([128, 1], F32)
            nc.scalar.activation(warm_ex, c_zero, AF.Exp)

            xp_sb = singles.tile([128, XPK], F32)
            nc.sync.dma_start(out=xp_sb, in_=xpin)
            e_sb = xp_sb[:, 0:P15]
            mrep_sb = xp_sb[:, P15:P15 + R]
            nbs_sb = xp_sb[:, XPK - 1:XPK]
            d2fmx_sb = singles.tile([128, FTOT], BF16)
            OFF8 = 4600  # offs[ABLK]
            nc.sync.dma_start(out=d2fmx_sb[:, OFF8:], in_=dfmin[:, OFF8:])
            nc.sync.dma_start(out=d2fmx_sb[:, :OFF8], in_=dfmin[:, :OFF8])

            # W1 DMA in 4 slices, descending chunk order
            w1sb = singles.tile([128, NCH, HID], BF16)
            for lo, hi in ((36, 47), (0, 12), (12, 24), (24, 36)):
                nc.sync.dma_start(
                    out=w1sb[:, lo:hi],
                    in_=w1s[:, lo * HID:hi * HID],
                )

            # PE warm-up: one long sustained burst to promote the clock
            ps_warm = ps_w_pool.tile([128, 128], BF16, tag="warm")
            for _ in range(NWARM):
                nc.tensor.transpose(ps_warm, ident, ident)

            # ------------- smearing g (two a-blocks, high block first) -------------
            t_sb = singles.tile([128, P15, R], F32)
            tsq_sb = singles.tile([128, P15, R], F32)
            g_sb = singles.tile([128, P15, R], BF16)
            for lo, hi in ((ABLK, P15), (0, ABLK)):
                nn_ = hi - lo
                nc.vector.tensor_sub(
                    t_sb[:, lo:hi],
                    _bcast(e_sb[:, lo:hi], 1, R),
                    _bcast(mrep_sb, 0, nn_),
                )
                nc.scalar.activation(tsq_sb[:, lo:hi], t_sb[:, lo:hi], AF.Square)
                nc.scalar.activation(
                    g_sb[:, lo:hi], tsq_sb[:, lo:hi], AF.Exp, scale=nbs_sb
                )

            # ---------------- att (dense 6016 cols, descending a) ----------------
            attb = singles.tile([128, FPAD], BF16)
            nc.gpsimd.memset(attb[:, FTOT:], 0.0)
            offs = []
            off = 0
            for a in range(P15):
                offs.append(off)
                off += (P15 - a) * R
            for a in list(range(ABLK, P15)) + list(range(ABLK)):
                cc = P15 - a
                g2_t = g2v.tile([128, cc, R], BF16, tag="g2")
                nc.vector.tensor_mul(
                    g2_t,
                    _with_dims(g_sb[:, a], [[0, cc], [1, R]]),
                    _with_dims(g_sb[:, a], [[R, cc], [1, R]]),
                )
                # flat bf16 mul (2x DVE mode): att = g2 * d2fmx, all step-1
                nc.vector.tensor_mul(
                    _with_dims(attb[:, offs[a]:], [[1, cc * R]]),
                    _with_dims(g2_t[:], [[1, cc * R]]),
                    _with_dims(d2fmx_sb[:, offs[a]:], [[1, cc * R]]),
                )

            # ---------------- transpose + matmul (descending chunks) ----------------
            attTb = singles.tile([128, NCH, 128], BF16)
            ps_acc = ps_acc_pool.tile([HID, B], F32)
            mm = 0
            prev = None

            def mm_group(chunk_list):
                nonlocal mm
                for c in chunk_list:
                    nc.tensor.matmul(
                        ps_acc,
                        lhsT=w1sb[:, c],
                        rhs=attTb[:, c],
                        start=(mm == 0),
                        stop=(mm == NCH - 1),
                    )
                    mm += 1

            for gi, grp in enumerate(groups):
                pst = ps_t_pool.tile([128, GRP, 128], BF16, tag="pst")
                c_lo = grp[0]
                for c in grp:
                    nc.tensor.transpose(
                        pst[:, c - c_lo], attb[:, c * 128:(c + 1) * 128], ident
                    )
                n_in = len(grp)
                if gi == len(groups) - 1:
                    nc.vector.tensor_copy(attTb[:, c_lo:c_lo + n_in], pst[:, :n_in])
                else:
                    nc.scalar.copy(attTb[:, c_lo:c_lo + n_in], pst[:, :n_in])
                if prev is not None:
                    mm_group(prev)
                prev = grp
            mm_group(prev)
            assert mm == NCH

            # ---------------- partial h1 out ----------------
            h1_sb = singles.tile([HID, B], F32)
            nc.scalar.copy(h1_sb, ps_acc)
            nc.sync.dma_start(out=h1outd, in_=h1_sb)

    nc.compile()
    return nc


def host_prep(x, W_map, means, betas, W1, b1, W2, b2):
    import ml_dtypes

    x = np.ascontiguousarray(np.asarray(x, np.float32))
    W_map = np.asarray(W_map, np.float32)
    means = np.asarray(means, np.float32)
    betas = np.asarray(betas, np.float32)
    W1 = np.asarray(W1, np.float32)

    # host prep (O(B*15) per head): coarse coords, pair geometry, cutoff
    xc_h = np.einsum('hkn,bnd->hbkd', W_map, x).astype(np.float64)  # (H,B,K,3)
    canon = [(i, j) for i in range(K) for j in range(i + 1, K)]
    ii = np.array([i for i, _ in canon])
    jj = np.array([j for _, j in canon])
    delta = xc_h[:, :, ii, :] - xc_h[:, :, jj, :]          # (H,B,15,3)
    d2 = (delta ** 2).sum(-1)
    dn = np.sqrt(d2)                                        # (H,B,15)
    e_h = np.exp(-dn)
    cutoff = 0.5 * (np.cos(dn * np.pi / CUT) + 1.0) * (dn < CUT)
    m3 = cutoff / (dn + 1e-6) ** 2                          # (H,B,15)
    d2f = np.einsum('hbad,hbcd->hbac', delta, delta)        # (H,B,15,15)
    d2fm = d2f * m3[:, :, :, None] * m3[:, :, None, :]      # (H,B,15,15)
    # expand over n into the dense (a, c>=a, n) layout matching att
    d2fmx = np.empty((H, B, FTOT), np.float32)
    off = 0
    for a in range(P15):
        cc = P15 - a
        d2fmx[:, :, off:off + cc * R] = np.repeat(
            d2fm[:, :, a, a:], R, axis=-1
        ).reshape(H, B, cc * R)
        off += cc * R
    d2fm_dev = np.ascontiguousarray(d2fmx.astype(ml_dtypes.bfloat16))

    # fold W1 onto the 120 triangular canonical pair-pairs
    P36 = K * K
    a_of = np.array([i * K + j for (i, j) in canon])
    abar = np.array([j * K + i for (i, j) in canon])
    W1r = W1.reshape(H, P36, P36, R, HID)
    W1q = (
        W1r[:, a_of[:, None], a_of[None, :]]
        - W1r[:, a_of[:, None], abar[None, :]]
        - W1r[:, abar[:, None], a_of[None, :]]
        + W1r[:, abar[:, None], abar[None, :]]
    )
    tri_a, tri_c = np.triu_indices(P15)
    W1t = W1q[:, tri_a, tri_c] + np.where(
        (tri_a != tri_c)[None, :, None, None], W1q[:, tri_c, tri_a], 0.0
    )
    W1flat = np.zeros((H, FPAD, HID), np.float32)
    W1flat[:, :FTOT] = W1t.reshape(H, FTOT, HID)
    W1s_dev = np.ascontiguousarray(
        W1flat.reshape(H, NCH, 128, HID).transpose(0, 2, 1, 3).reshape(H, 128, NCH * HID)
        .astype(ml_dtypes.bfloat16)
    )

    assert np.all(betas == betas[0]), "kernel folds the uniform beta into Exp"
    xpack = np.zeros((H, B, XPK), np.float32)
    xpack[:, :, :P15] = e_h
    xpack[:, :, P15:P15 + R] = means[None, None, :]
    xpack[:, :, XPK - 1] = -float(betas[0])

    return [
        dict(
            xpin=np.ascontiguousarray(xpack[h]),
            dfmin=d2fm_dev[h],
            w1s=W1s_dev[h],
        )
        for h in range(H)
    ]


_NC_CACHE = {}


def get_program():
    if "nc" not in _NC_CACHE:
        _NC_CACHE["nc"] = build_program()
    return _NC_CACHE["nc"]


def kernel(x, W_map, means, betas, W1, b1, W2, b2, _debug=False, _trace=False):
    in_maps = host_prep(x, W_map, means, betas, W1, b1, W2, b2)
    nc = get_program()
    res = run_bass_kernel_spmd(nc, in_maps, list(range(H)), trace=_trace)
    h1 = np.zeros((HID, B), np.float64)
    for r in res.results:
        h1 += np.asarray(r["h1out"], np.float32)
    b1 = np.asarray(b1, np.float64).reshape(HID, 1)
    W2v = np.asarray(W2, np.float64).reshape(HID)
    z = h1 + b1
    sig = 1.0 / (1.0 + np.exp(-z))
    out = (W2v @ (z * sig)) + float(np.asarray(b2).reshape(()))
    if _debug or _trace:
        kernel.last_results = res
    return out[:, None].astype(np.float32)


# revision 8
# speedup vs baseline: 1.4411x; 1.0929x over previous
"""Trainium2 Bass kernel for nn_LilletLayer (gnn_message_passing) — v7.

Math per head h, molecule b (reference-exact algebra):
  att[a,c,n] = D2[a,c] * g[a,n] * g[c,n] over 15 canonical pairs, folded
  to 120 triangular pair-pairs (6000 rows); h1_h = W1_h^T att_h.

Split:
 - HOST (O(B*15) trivial prep, like the existing xc fold): coarse coords
   xc, pair deltas, distances, cutoff, 1/(d+1e-6)^2, D2 gram — shipped as
   e[b,a]=exp(-d) and d2fm[b,a,c]=D2*cutoff_a*cutoff_c/(d_a d_c)^2 (bf16).
 - DEVICE (one core per head): the O(B*6000) smearing g = exp(-beta*
   (e-mu)^2) (ACT Square+Exp — a single activation-table set, warmed at
   t=0), the 1.5M-element att outer products (DVE), 47 PE transposes and
   47 [128x128x128] bf16 matmuls accumulating h1_h in PSUM fp32.
 - HOST: sum the 8 h1 partials + silu + W2 + b2 (49 kFLOP). No device
   collective — an 8-rank AllReduce of 32KB costs ~50us wall here (mesh
   latency + launch skew) and is intermittently flaky.

Scheduling: g in two a-blocks (high first), the per-a att loop runs
descending a, transposes/copies/matmuls and the W1 DMA follow the same
descending order; one long PE warm burst promotes the PE clock before
the real transposes; PSUM->SBUF copies split ACT/DVE.
"""

import numpy as np

import concourse.bacc as bacc
import concourse.bass as bass
import concourse.mybir as mybir
import concourse.tile as tile
from concourse.bass_utils import run_bass_kernel_spmd
from concourse.masks import make_identity

B, N, H, K, R = 128, 512, 8, 6, 50
CUT = 5.0
P15 = K * (K - 1) // 2
NPAIR = P15 * (P15 + 1) // 2
FTOT = NPAIR * R              # 6000
NCH = 47
FPAD = NCH * 128              # 6016
HID = 128
XPK = P15 + R + 1             # packed fp32 input: e(15) means(50) nbs(1)
F32 = mybir.dt.float32
BF16 = mybir.dt.bfloat16
AF = mybir.ActivationFunctionType
ALU = mybir.AluOpType

GBLKS = ((10, 15), (5, 10), (0, 5))  # g blocks, high a first
GRP = 6       # transpose chunks per PSUM group
NWARM = 48
NDVE_COPY = 2  # trailing copy groups on DVE


def _bcast(ap, axis, count):
    dims = [list(d) for d in ap.ap]
    dims.insert(axis + 1, [0, count])
    return bass.AP(tensor=ap.tensor, offset=ap.offset, ap=dims)


def _with_dims(ap, dims):
    return bass.AP(
        tensor=ap.tensor, offset=ap.offset, ap=[list(ap.ap[0])] + [list(d) for d in dims]
    )


def build_program(n_cores=8):
    nc = bacc.Bacc(
        "TRN2",
        target_bir_lowering=False,
        debug=False,
        enable_asserts=False,
        num_devices=n_cores,
    )

    xpin = nc.dram_tensor("xpin", [B, XPK], F32, kind="ExternalInput").ap()
    dfmin = nc.dram_tensor("dfmin", [B, FTOT], BF16, kind="ExternalInput").ap()
    w1s = nc.dram_tensor("w1s", [128, NCH * HID], BF16, kind="ExternalInput").ap()
    h1outd = nc.dram_tensor("h1out", [HID, B], F32, kind="ExternalOutput").ap()

    # block a>=10 covers cols [5250,6016) -> full chunks 42..46;
    # block a>=5 covers cols >=3250 -> chunks 26..41; a<5 -> chunks 0..25.
    groups = ([list(range(42, 47))]
              + [list(range(26, 32)), list(range(32, 38)), list(range(38, 42))]
              + [list(range(i, min(i + GRP, 26))) for i in range(0, 26, GRP)])

    with tile.TileContext(nc) as tc:
        with (
            tc.tile_pool(name="singles", bufs=1) as singles,
            tc.tile_pool(name="g2v", bufs=2) as g2v,
            tc.tile_pool(name="ps_t", bufs=4, space="PSUM") as ps_t_pool,
            tc.tile_pool(name="ps_acc", bufs=1, space="PSUM") as ps_acc_pool,
            tc.tile_pool(name="ps_w", bufs=1, space="PSUM") as ps_w_pool,
        ):
            # ---------------- t=0: DMAs, table warm, identity ----------------
            ident = singles.tile([128, 128], BF16)
            make_identity(nc, ident)

            c_zero = singles.tile([128, 1], F32)
            nc.vector.memset(c_zero, 0.0)
            warm_ex = singles.tile([128, 1], F32)
            nc.scalar.activation(warm_ex, c_zero, AF.Exp)

            xp_sb = singles.tile([128, XPK], F32)
            nc.sync.dma_start(out=xp_sb, in_=xpin)
            e_sb = xp_sb[:, 0:P15]
            mrep_sb = xp_sb[:, P15:P15 + R]
            nbs_sb = xp_sb[:, XPK - 1:XPK]
            d2fmx_sb = singles.tile([128, FTOT], BF16)
            nc.sync.dma_start(out=d2fmx_sb[:, 5250:], in_=dfmin[:, 5250:])
            nc.sync.dma_start(out=d2fmx_sb[:, 3250:5250], in_=dfmin[:, 3250:5250])
            nc.sync.dma_start(out=d2fmx_sb[:, :3250], in_=dfmin[:, :3250])

            # W1 DMA in 4 slices, descending chunk order
            w1sb = singles.tile([128, NCH, HID], BF16)
            for lo, hi in ((42, 47), (26, 42), (0, 13), (13, 26)):
                nc.sync.dma_start(
                    out=w1sb[:, lo:hi],
                    in_=w1s[:, lo * HID:hi * HID],
                )

            # PE warm-up: one long sustained burst to promote the clock
            ps_warm = ps_w_pool.tile([128, 128], BF16, tag="warm")
            for _ in range(NWARM):
                nc.tensor.transpose(ps_warm, ident, ident)

            # ------------- smearing g (two a-blocks, high block first) -------------
            t_sb = singles.tile([128, P15, R], F32)
            tsq_sb = singles.tile([128, P15, R], F32)
            g_sb = singles.tile([128, P15, R], BF16)
            for lo, hi in GBLKS:
                nn_ = hi - lo
                nc.vector.tensor_sub(
                    t_sb[:, lo:hi],
                    _bcast(e_sb[:, lo:hi], 1, R),
                    _bcast(mrep_sb, 0, nn_),
                )
                nc.scalar.activation(tsq_sb[:, lo:hi], t_sb[:, lo:hi], AF.Square)
                nc.scalar.activation(
                    g_sb[:, lo:hi], tsq_sb[:, lo:hi], AF.Exp, scale=nbs_sb
                )

            # ---------------- att (dense 6016 cols, descending a) ----------------
            attb = singles.tile([128, FPAD], BF16)
            nc.gpsimd.memset(attb[:, FTOT:], 0.0)
            offs = []
            off = 0
            for a in range(P15):
                offs.append(off)
                off += (P15 - a) * R
            for a in [a for lo, hi in GBLKS for a in range(lo, hi)]:
                cc = P15 - a
                g2_t = g2v.tile([128, cc, R], BF16, tag="g2")
                nc.vector.tensor_mul(
                    g2_t,
                    _with_dims(g_sb[:, a], [[0, cc], [1, R]]),
                    _with_dims(g_sb[:, a], [[R, cc], [1, R]]),
                )
                # flat bf16 mul (2x DVE mode): att = g2 * d2fmx, all step-1
                nc.vector.tensor_mul(
                    _with_dims(attb[:, offs[a]:], [[1, cc * R]]),
                    _with_dims(g2_t[:], [[1, cc * R]]),
                    _with_dims(d2fmx_sb[:, offs[a]:], [[1, cc * R]]),
                )

            # ---------------- transpose + matmul (descending chunks) ----------------
            attTb = singles.tile([128, NCH, 128], BF16)
            ps_acc = ps_acc_pool.tile([HID, B], F32)
            mm = 0
            prev = None

            def mm_group(chunk_list):
                nonlocal mm
                for c in chunk_list:
                    nc.tensor.matmul(
                        ps_acc,
                        lhsT=w1sb[:, c],
                        rhs=attTb[:, c],
                        start=(mm == 0),
                        stop=(mm == NCH - 1),
                    )
                    mm += 1

            for gi, grp in enumerate(groups):
                pst = ps_t_pool.tile([128, GRP, 128], BF16, tag="pst")
                c_lo = grp[0]
                for c in grp:
                    nc.tensor.transpose(
                        pst[:, c - c_lo], attb[:, c * 128:(c + 1) * 128], ident
                    )
                n_in = len(grp)
                tail4 = gi >= len(groups) - 4
                if tail4 and (len(groups) - 1 - gi) % 2 == 0:
                    nc.vector.tensor_copy(attTb[:, c_lo:c_lo + n_in], pst[:, :n_in])
                else:
                    nc.scalar.copy(attTb[:, c_lo:c_lo + n_in], pst[:, :n_in])
                if prev is not None:
                    mm_group(prev)
                prev = grp
            mm_group(prev)
            assert mm == NCH

            # ---------------- partial h1 out ----------------
            h1_sb = singles.tile([HID, B], F32)
            nc.vector.tensor_copy(h1_sb, ps_acc)
            nc.sync.dma_start(out=h1outd, in_=h1_sb)

    nc.compile()
    return nc


def host_prep(x, W_map, means, betas, W1, b1, W2, b2):
    import ml_dtypes

    x = np.ascontiguousarray(np.asarray(x, np.float32))
    W_map = np.asarray(W_map, np.float32)
    means = np.asarray(means, np.float32)
    betas = np.asarray(betas, np.float32)
    W1 = np.asarray(W1, np.float32)

    # host prep (O(B*15) per head): coarse coords, pair geometry, cutoff
    xc_h = np.einsum('hkn,bnd->hbkd', W_map, x).astype(np.float64)  # (H,B,K,3)
    canon = [(i, j) for i in range(K) for j in range(i + 1, K)]
    ii = np.array([i for i, _ in canon])
    jj = np.array([j for _, j in canon])
    delta = xc_h[:, :, ii, :] - xc_h[:, :, jj, :]          # (H,B,15,3)
    d2 = (delta ** 2).sum(-1)
    dn = np.sqrt(d2)                                        # (H,B,15)
    e_h = np.exp(-dn)
    cutoff = 0.5 * (np.cos(dn * np.pi / CUT) + 1.0) * (dn < CUT)
    m3 = cutoff / (dn + 1e-6) ** 2                          # (H,B,15)
    d2f = np.einsum('hbad,hbcd->hbac', delta, delta)        # (H,B,15,15)
    d2fm = d2f * m3[:, :, :, None] * m3[:, :, None, :]      # (H,B,15,15)
    # expand over n into the dense (a, c>=a, n) layout matching att
    d2fmx = np.empty((H, B, FTOT), np.float32)
    off = 0
    for a in range(P15):
        cc = P15 - a
        d2fmx[:, :, off:off + cc * R] = np.repeat(
            d2fm[:, :, a, a:], R, axis=-1
        ).reshape(H, B, cc * R)
        off += cc * R
    d2fm_dev = np.ascontiguousarray(d2fmx.astype(ml_dtypes.bfloat16))

    # fold W1 onto the 120 triangular canonical pair-pairs
    P36 = K * K
    a_of = np.array([i * K + j for (i, j) in canon])
    abar = np.array([j * K + i for (i, j) in canon])
    W1r = W1.reshape(H, P36, P36, R, HID)
    W1q = (
        W1r[:, a_of[:, None], a_of[None, :]]
        - W1r[:, a_of[:, None], abar[None, :]]
        - W1r[:, abar[:, None], a_of[None, :]]
        + W1r[:, abar[:, None], abar[None, :]]
    )
    tri_a, tri_c = np.triu_indices(P15)
    W1t = W1q[:, tri_a, tri_c] + np.where(
        (tri_a != tri_c)[None, :, None, None], W1q[:, tri_c, tri_a], 0.0
    )
    W1flat = np.zeros((H, FPAD, HID), np.float32)
    W1flat[:, :FTOT] = W1t.reshape(H, FTOT, HID)
    W1s_dev = np.ascontiguousarray(
        W1flat.reshape(H, NCH, 128, HID).transpose(0, 2, 1, 3).reshape(H, 128, NCH * HID)
        .astype(ml_dtypes.bfloat16)
    )

    assert np.all(betas == betas[0]), "kernel folds the uniform beta into Exp"
    xpack = np.zeros((H, B, XPK), np.float32)
    xpack[:, :, :P15] = e_h
    xpack[:, :, P15:P15 + R] = means[None, None, :]
    xpack[:, :, XPK - 1] = -float(betas[0])

    return [
        dict(
            xpin=np.ascontiguousarray(xpack[h]),
            dfmin=d2fm_dev[h],
            w1s=W1s_dev[h],
        )
        for h in range(H)
    ]


_NC_CACHE = {}


def get_program():
    if "nc" not in _NC_CACHE:
        _NC_CACHE["nc"] = build_program()
    return _NC_CACHE["nc"]


def kernel(x, W_map, means, betas, W1, b1, W2, b2, _debug=False, _trace=False):
    in_maps = host_prep(x, W_map, means, betas, W1, b1, W2, b2)
    nc = get_program()
    res = run_bass_kernel_spmd(nc, in_maps, list(range(H)), trace=_trace)
    h1 = np.zeros((HID, B), np.float64)
    for r in res.results:
        h1 += np.asarray(r["h1out"], np.float32)
    b1 = np.asarray(b1, np.float64).reshape(HID, 1)
    W2v = np.asarray(W2, np.float64).reshape(HID)
    z = h1 + b1
    sig = 1.0 / (1.0 + np.exp(-z))
    out = (W2v @ (z * sig)) + float(np.asarray(b2).reshape(()))
    if _debug or _trace:
        kernel.last_results = res
    return out[:, None].astype(np.float32)
